# revision 38
# baseline (speedup 1.0000x reference)
"""Trainium2 Bass kernel for nn_Attention (S=2048, B=2, D=1024, H=16, C=64).

Tensor-parallel over heads across 8 NeuronCores (2 heads/core), fully
interleaved wavefront:
  - All static inputs are pre-cast to bf16 on the host, so every load is
    a plain (non-casting) DMA and queue assignment is free.
  - p1 (projections+norm+rope, 8 chunks of 512 tokens): Wq/Wk pre-scaled
    by the RMSNorm weights on host; sumsq of the raw projection recovered
    via matmul against 1/w^2 selector columns; Sqrt runs in two batched
    regions (2 ACT table swaps total against the softmax Exp); the
    reciprocal runs on a DRAM-reshaped [64,32] view; rstd returns as a
    stride-0 broadcast DMA and is applied as the last multiply (rope
    commutes with the per-token scale); the rope pair-swap is a PE
    permutation matmul. The elementwise chain is bf16 for the DVE 2x
    mode, split across DVE and GpSimd.
  - p2 (attention, 8 query chunks of 512): scores transposed [keys, q]
    per head with K=64 contraction on PE quadrants; one [128,1024] exp
    per key block covers both heads; attn@v accumulates in PSUM with an
    appended ones column so the softmax denominator falls out.
  - The AllToAll re-shard runs at chunk granularity (8 collectives,
    destination core = (token//64) % 8), each fired right after its
    chunk; collectives are issued from the Sync queue (a collective
    trigger blocks its issuing queue until completion, so nothing
    latency-critical may sit behind it — GpSimd keeps the bounce DMAs
    and rope arithmetic instead). p3 out-projection runs per received
    eighth, two chunks after its collective fired. p1 pieces and p3
    eighths drain from an interleave queue between p2 key blocks so the
    PE stays continuously busy (p-state ramp) while ACT paces the exps.
"""

import sys

if "/opt/trn_rl_repo" not in sys.path:
    sys.path.insert(0, "/opt/trn_rl_repo")

import numpy as np
import ml_dtypes
import concourse.bass as bass
from concourse import bacc, tile, mybir
from concourse.bass_utils import run_bass_kernel_spmd
from concourse.masks import make_identity

S, B, D, H, C = 2048, 2, 1024, 16, 64
EPS = 1e-6
NCORES = 8
T = S * B                  # 4096 tokens, batch-major: t = b*S + s
LH = H // NCORES           # 2 local heads
LC = LH * C                # 128 local head columns
TCH = 512                  # p1/p2 token chunk
NCH = T // TCH             # 8
NJT = S // 128             # 16 key blocks per batch
TOK_OUT = T // NCORES      # 512 output tokens per core

F32 = mybir.dt.float32
BF16 = mybir.dt.bfloat16
AF = mybir.ActivationFunctionType

_CACHE = {}
LAST_RESULTS = None


def _build():
    nc = bacc.Bacc("TRN2", target_bir_lowering=False, debug=False,
                   num_devices=NCORES)
    xT = nc.dram_tensor("xT", [D, T], BF16, kind="ExternalInput")
    wqkv = nc.dram_tensor("wqkv", [D, 3 * LC], BF16, kind="ExternalInput")
    wo = nc.dram_tensor("wo", [H * C, D], BF16, kind="ExternalInput")
    ropeAB = nc.dram_tensor("ropeAB", [LC, 2 * T], BF16,
                            kind="ExternalInput")
    winv = nc.dram_tensor("winv", [128, 8], BF16, kind="ExternalInput")
    perm = nc.dram_tensor("perm", [128, 128], BF16, kind="ExternalInput")
    out = nc.dram_tensor("out", [TOK_OUT, D], F32, kind="ExternalOutput")

    xT4 = xT.rearrange("(a p) t -> p a t", p=128)        # [128, 8, T]
    wqkv4 = wqkv.rearrange("(a p) c -> p a c", p=128)    # [128, 8, 3*LC]
    wo4 = wo.rearrange("(a p) n -> p a n", p=128)        # [128, 8, D]
    rope4 = ropeAB.rearrange("p (j t) -> p j t", j=2)    # [128, 2, T]

    with tile.TileContext(nc) as tc:
        with (
            tc.tile_pool(name="singles", bufs=1) as singles,
            tc.tile_pool(name="xtp", bufs=4) as xtp,
            tc.tile_pool(name="workp", bufs=2) as workp,
            tc.tile_pool(name="ps1", bufs=2, space="PSUM") as ps1p,
            tc.tile_pool(name="pss", bufs=2, space="PSUM") as pssp,
            tc.tile_pool(name="pos", bufs=2, space="PSUM") as posp,
            tc.tile_pool(name="dram", bufs=1, space="DRAM") as dram,
        ):
            # ---- constants ----
            ident = singles.tile([128, 128], BF16)
            make_identity(nc, ident)
            winv_sb = singles.tile([128, 8], BF16)
            nc.gpsimd.dma_start(out=winv_sb, in_=winv[:, :])
            permb = singles.tile([128, 128], BF16)
            nc.gpsimd.dma_start(out=permb, in_=perm[:, :])
            eps128 = singles.tile([128, 1], F32)
            nc.vector.memset(eps128, EPS)
            rdr_seed = singles.tile([NCORES, 64], BF16)
            nc.vector.memset(rdr_seed, 0.0)

            # ---- weights / rope (wo deferred: only p3 needs it) ----
            wqkv_sb = singles.tile([128, 8, 3 * LC], BF16)
            nc.gpsimd.dma_start(out=wqkv_sb, in_=wqkv4)
            rope_sb = singles.tile([128, 2, T], BF16)
            nc.gpsimd.dma_start(out=rope_sb[:, :, 0:T // 2],
                                in_=rope4[:, :, 0:T // 2])
            nc.gpsimd.dma_start(out=rope_sb[:, :, T // 2:T],
                                in_=rope4[:, :, T // 2:T])
            wo_sb = singles.tile([128, 8, D], BF16)

            # ---- persistent activations ----
            qT_sb = singles.tile([128, T], BF16)   # [2 heads x 64c, t]
            k_sb = singles.tile([128, T], BF16)
            v_sb = singles.tile([128, T // 128, LH, C + 1], BF16)
            onescol = singles.tile([128, T // 128, LH, 1], F32)
            nc.vector.memset(onescol, 1.0)
            nc.vector.tensor_copy(v_sb[:, :, :, C:C + 1], onescol)

            wbin = dram.tile([NCORES, 64], BF16, tag="wbin", name="wbin")
            wbout = dram.tile([NCORES, 64], BF16, tag="wbout", name="wbout")
            bins = [dram.tile([NCORES * 128, 64], BF16, tag=f"bin{q}",
                              name=f"bin{q}") for q in range(NCH)]
            bouts = [dram.tile([NCORES * 128, 64], BF16, tag=f"bout{q}",
                               name=f"bout{q}") for q in range(NCH)]

            xts = {}
            st = {ch: {} for ch in range(NCH)}

            def load_xt(ch):
                xt = xtp.tile([128, 8, TCH], BF16, tag="xt", name=f"xt{ch}")
                nc.gpsimd.dma_start(out=xt, in_=xT4[:, :, ch * TCH:
                                                    (ch + 1) * TCH])
                xts[ch] = xt

            # ---- p1 pieces (ACT used only for exp/sqrt) ----
            def p1_q(ch):
                if ch + 3 < NCH:
                    load_xt(ch + 3)
                psq = ps1p.tile([128, TCH], F32, tag="ps1", name=f"psq{ch}")
                for a in range(8):
                    nc.tensor.matmul(psq, wqkv_sb[:, a, 0:LC],
                                     xts[ch][:, a, :],
                                     start=(a == 0), stop=(a == 7))
                qraw = workp.tile([128, TCH], BF16, tag="qraw", bufs=6,
                                  name=f"qraw{ch}")
                nc.vector.tensor_copy(qraw, psq)
                sq2 = workp.tile([128, TCH], BF16, tag="sq2",
                                 name=f"sq2_{ch}")
                nc.vector.tensor_mul(sq2, qraw, qraw)
                st[ch]["qraw"], st[ch]["sq2"] = qraw, sq2

            def p1_k(ch):
                psk = ps1p.tile([128, TCH], F32, tag="ps1", name=f"psk{ch}")
                for a in range(8):
                    nc.tensor.matmul(psk, wqkv_sb[:, a, LC:2 * LC],
                                     xts[ch][:, a, :],
                                     start=(a == 0), stop=(a == 7))
                kraw = workp.tile([128, TCH], BF16, tag="kraw", bufs=6,
                                  name=f"kraw{ch}")
                nc.vector.tensor_copy(kraw, psk)
                sqk2 = workp.tile([128, TCH], BF16, tag="sqk2",
                                  name=f"sqk2_{ch}")
                nc.vector.tensor_mul(sqk2, kraw, kraw)
                st[ch]["kraw"], st[ch]["sqk2"] = kraw, sqk2

            def p1_v(ch):
                t0 = ch * TCH
                psv = ps1p.tile([128, TCH], F32, tag="ps1", name=f"psv{ch}")
                for a in range(8):
                    nc.tensor.matmul(psv, wqkv_sb[:, a, 2 * LC:3 * LC],
                                     xts[ch][:, a, :],
                                     start=(a == 0), stop=(a == 7))
                vt = workp.tile([128, TCH], BF16, tag="vt", name=f"vt{ch}")
                nc.vector.tensor_copy(vt, psv)
                ptv = ps1p.tile([128, 4, 128], BF16, tag="ps1",
                                name=f"ptv{ch}")
                for s5 in range(TCH // 128):
                    nc.tensor.transpose(ptv[:, s5, :],
                                        vt[:, s5 * 128:(s5 + 1) * 128],
                                        ident)
                blk0 = t0 // 128
                nc.vector.tensor_copy(
                    v_sb[:, blk0:blk0 + 4, :, 0:C],
                    ptv.rearrange("p f (l c) -> p f l c", l=LH))

            def p1_ms(ch):
                # sumsq rows 0:2 = q heads, 2:4 = k heads (one bank)
                ms = ps1p.tile([4, TCH], F32, tag="ps1", name=f"ms{ch}")
                nc.tensor.matmul(ms, winv_sb[:, 0:4], st[ch]["sq2"],
                                 start=True, stop=False)
                nc.tensor.matmul(ms, winv_sb[:, 4:8], st[ch]["sqk2"],
                                 start=False, stop=True)
                msb = workp.tile([4, TCH], F32, tag="msb", bufs=6,
                                 name=f"msb{ch}")
                nc.scalar.copy(msb, ms)
                st[ch]["msb"] = msb

            def p1_sqrt(ch):
                std = workp.tile([4, TCH], F32, tag="std", bufs=6,
                                 name=f"std{ch}")
                nc.scalar.activation(std, st[ch]["msb"], AF.Sqrt,
                                     bias=eps128[0:4, :], scale=1.0 / C)
                st[ch]["std"] = std

            def p1_fin(ch):
                t0 = ch * TCH
                rdr = dram.tile([4, TCH], F32, tag="rdr", bufs=2,
                                name=f"rdr{ch}")
                nc.sync.dma_start(out=rdr, in_=st[ch]["std"])
                rsh = workp.tile([64, 4 * TCH // 64], F32, tag="rsh",
                                 name=f"rsh{ch}")
                nc.sync.dma_start(
                    out=rsh,
                    in_=bass.AP(tensor=rdr.tensor, offset=rdr.offset,
                                ap=[[4 * TCH // 64, 64], [1, 4 * TCH // 64]]))
                rshr = workp.tile([64, 4 * TCH // 64], F32, tag="rshr",
                                  name=f"rshr{ch}")
                nc.vector.reciprocal(rshr, rsh)
                rdr2 = dram.tile([4, TCH], F32, tag="rdr2", bufs=2,
                                 name=f"rdr2{ch}")
                nc.sync.dma_start(
                    out=bass.AP(tensor=rdr2.tensor, offset=rdr2.offset,
                                ap=[[4 * TCH // 64, 64], [1, 4 * TCH // 64]]),
                    in_=rshr)
                bqw = workp.tile([128, TCH], F32, tag="bqw",
                                 name=f"bqw{ch}")
                nc.sync.dma_start(
                    out=bqw,
                    in_=bass.AP(tensor=rdr2.tensor, offset=rdr2.offset,
                                ap=[[TCH, 2], [0, 64], [1, TCH]]))
                bkw = workp.tile([128, TCH], F32, tag="bkw",
                                 name=f"bkw{ch}")
                nc.sync.dma_start(
                    out=bkw,
                    in_=bass.AP(tensor=rdr2.tensor,
                                offset=rdr2.offset + 2 * TCH,
                                ap=[[TCH, 2], [0, 64], [1, TCH]]))
                # rope + deferred rstd multiply (bf16 chain)
                for which, raw, bw, dst in (
                        ("q", st[ch]["qraw"], bqw, qT_sb),
                        ("k", st[ch]["kraw"], bkw, k_sb)):
                    t1 = workp.tile([128, TCH], BF16, tag="t1",
                                    name=f"t1{which}{ch}")
                    nc.vector.tensor_mul(t1, rope_sb[:, 0, t0:t0 + TCH], raw)
                    rot = ps1p.tile([128, TCH], F32, tag="ps1",
                                    name=f"rot{which}{ch}")
                    nc.tensor.matmul(rot, permb, raw, start=True, stop=True)
                    t2 = workp.tile([128, TCH], BF16, tag="t2",
                                    name=f"t2{which}{ch}")
                    nc.vector.tensor_mul(t2, rope_sb[:, 1, t0:t0 + TCH], rot)
                    t3 = workp.tile([128, TCH], BF16, tag="t3",
                                    name=f"t3{which}{ch}")
                    nc.gpsimd.tensor_add(t3, t1, t2)
                    nc.gpsimd.tensor_mul(dst[:, t0:t0 + TCH], t3, bw)

            # =============== phase-2 =====================================
            pos_tiles = {}

            p2_exs = {}

            def p2_part(c, r0, r1, interleave, pop_every=False):
                b = c // 4
                q0 = c * TCH
                if c not in pos_tiles:
                    pos_tiles[c] = [posp.tile([C + 1, TCH], F32, tag="pos",
                                              name=f"pos{c}_{lh}")
                                    for lh in range(LH)]
                    p2_exs[c] = {}
                exs = p2_exs[c]

                def scores_exp(jt):
                    j0 = b * S + jt * 128
                    pss = pssp.tile([128, LH, TCH], F32, tag="pss",
                                    name=f"pss{c}_{jt}")
                    for lh in range(LH):
                        nc.tensor.matmul(
                            pss[:, lh, :],
                            k_sb[64 * lh:64 * lh + 64, j0:j0 + 128],
                            qT_sb[64 * lh:64 * lh + 64, q0:q0 + TCH],
                            start=True, stop=True)
                    ex = workp.tile([128, LH, TCH], BF16, tag="ex", bufs=5,
                                    name=f"ex{c}_{jt}")
                    nc.scalar.activation(ex, pss, AF.Exp, bias=0.0,
                                         scale=0.125)
                    exs[jt] = ex

                def attnv(jt):
                    j0 = b * S + jt * 128
                    for lh in range(LH):
                        nc.tensor.matmul(
                            pos_tiles[c][lh],
                            v_sb[:, j0 // 128, lh, :],
                            exs[jt][:, lh, :],
                            start=(jt == 0), stop=(jt == NJT - 1))

                for r in range(r0, r1):
                    scores_exp(2 * r)
                    scores_exp(2 * r + 1)
                    if r >= 1:
                        attnv(2 * r - 2)
                        attnv(2 * r - 1)
                    if interleave and (pop_every or r % 2 == 1):
                        interleave.pop(0)()
                if r1 < NJT // 2:
                    return
                attnv(NJT - 2)
                attnv(NJT - 1)
                if interleave:
                    interleave.pop(0)()
                # --- normalize + bounce ---
                den2 = workp.tile([128, 2, TCH], F32, tag="den2",
                                  name=f"den2_{c}")
                for lh in range(LH):
                    nc.vector.tensor_copy(den2[C:C + 1, lh, :],
                                          pos_tiles[c][lh][C:C + 1, :])
                ddr = dram.tile([2, TCH], F32, tag="ddr", bufs=2,
                                name=f"ddr{c}")
                nc.sync.dma_start(out=ddr, in_=den2[C:C + 1, :, :])
                dsh = workp.tile([64, 2 * TCH // 64], F32, tag="dsh",
                                 name=f"dsh{c}")
                nc.sync.dma_start(
                    out=dsh,
                    in_=bass.AP(tensor=ddr.tensor, offset=ddr.offset,
                                ap=[[2 * TCH // 64, 64], [1, 2 * TCH // 64]]))
                dshr = workp.tile([64, 2 * TCH // 64], F32, tag="dshr",
                                  name=f"dshr{c}")
                nc.vector.reciprocal(dshr, dsh)
                ddr2 = dram.tile([2, TCH], F32, tag="ddr2", bufs=2,
                                 name=f"ddr2{c}")
                nc.sync.dma_start(
                    out=bass.AP(tensor=ddr2.tensor, offset=ddr2.offset,
                                ap=[[2 * TCH // 64, 64], [1, 2 * TCH // 64]]),
                    in_=dshr)
                for lh in range(LH):
                    dnb = workp.tile([C, TCH], F32, tag="dnb",
                                     name=f"dnb{c}_{lh}")
                    nc.sync.dma_start(
                        out=dnb,
                        in_=bass.AP(tensor=ddr2.tensor,
                                    offset=ddr2.offset + lh * TCH,
                                    ap=[[0, C], [1, TCH]]))
                    attbf = workp.tile([C, TCH], BF16, tag="attbf",
                                       name=f"attbf{c}_{lh}")
                    nc.vector.tensor_mul(attbf,
                                         pos_tiles[c][lh][0:C, :], dnb)
                    # rows d*128 + 64*lh + p, 64-token slices per dest
                    nc.gpsimd.dma_start(
                        out=bass.AP(
                            tensor=bins[c].tensor,
                            offset=bins[c].offset + C * lh * 64,
                            ap=[[64, C], [128 * 64, 8], [1, 64]]),
                        in_=attbf.rearrange("p (d t) -> p d t", d=8))

            def collective(q):
                nc.gpsimd.collective_compute(
                    "AllToAll", mybir.AluOpType.bypass,
                    replica_groups=[list(range(NCORES))],
                    ins=[bins[q][:, :].opt()],
                    outs=[bouts[q][:, :].opt()])

            # =============== phase-3 (per eighth) ========================
            def p3_eighth(q):
                atta = workp.tile([128, 8, 64], BF16, tag="atta",
                                  name=f"atta{q}")
                nc.sync.dma_start(
                    out=atta,
                    in_=bouts[q].rearrange("(g p) t -> p g t", p=128))
                for nh in range(2):
                    po3 = ps1p.tile([64, 512], F32, tag="ps1",
                                    name=f"po3_{q}_{nh}")
                    for a in range(8):
                        nc.tensor.matmul(
                            po3, atta[:, a, :],
                            wo_sb[:, a, nh * 512:(nh + 1) * 512],
                            start=(a == 0), stop=(a == 7))
                    outsb = workp.tile([64, 512], F32, tag="outsb",
                                       name=f"outsb{q}_{nh}")
                    nc.vector.tensor_copy(outsb, po3)
                    nc.sync.dma_start(
                        out=out[q * 64:(q + 1) * 64,
                                nh * 512:(nh + 1) * 512],
                        in_=outsb)

            # =============== schedule ====================================
            load_xt(0)
            load_xt(1)
            load_xt(2)

            for pair in (0, 2):
                for ch in (pair, pair + 1):
                    p1_q(ch)
                    p1_k(ch)
                    p1_v(ch)
                    p1_ms(ch)
                p1_sqrt(pair)
                p1_sqrt(pair + 1)
                p1_fin(pair)
                p1_fin(pair + 1)
            # warm up the collective path: the first AllToAll pays ~35us of
            # one-time setup; burn it on a tiny dummy while p2 ramps up
            nc.sync.dma_start(out=wbin, in_=rdr_seed)
            nc.gpsimd.collective_compute(
                "AllToAll", mybir.AluOpType.bypass,
                replica_groups=[list(range(NCORES))],
                ins=[wbin[:, :].opt()], outs=[wbout[:, :].opt()])
            nc.gpsimd.dma_start(out=wo_sb, in_=wo4)  # deferred load

            ilq = []
            for pair in (4, 6):
                for ch in (pair, pair + 1):
                    ilq.append(lambda ch=ch: p1_q(ch))
                    ilq.append(lambda ch=ch: p1_k(ch))
                    ilq.append(lambda ch=ch: p1_v(ch))
                    ilq.append(lambda ch=ch: p1_ms(ch))

                def sqrt_pair(pair=pair):
                    p1_sqrt(pair)
                    p1_sqrt(pair + 1)
                ilq.append(sqrt_pair)
                ilq.append(lambda ch=pair: p1_fin(ch))
                ilq.append(lambda ch=pair + 1: p1_fin(ch))

            for c in range(NCH):
                p2_part(c, 0, NJT // 2, ilq, pop_every=(c < 4))
                collective(c)
                if c >= 2:
                    ilq.append(lambda c=c: p3_eighth(c - 2))
            while ilq:
                ilq.pop(0)()
            p3_eighth(NCH - 2)
            p3_eighth(NCH - 1)

    nc.compile()
    return nc


def kernel(x, rope_emb, Wq, Wk, Wv, q_norm_w, k_norm_w, Wout):
    global LAST_RESULTS
    if "nc" not in _CACHE:
        _CACHE["nc"] = _build()
    nc = _CACHE["nc"]
    bf16 = ml_dtypes.bfloat16

    # batch-major tokens: t = b*S + s
    x2 = np.ascontiguousarray(
        np.transpose(np.asarray(x, np.float32), (1, 0, 2)).reshape(T, D))
    xT_np = np.ascontiguousarray(x2.T.astype(bf16))

    re = np.asarray(rope_emb, np.float32)
    cosT = np.ascontiguousarray(re[:, :, 0, 0].T)    # [32, S]
    r01T = np.ascontiguousarray(re[:, :, 0, 1].T)
    r10T = np.ascontiguousarray(re[:, :, 1, 0].T)
    cos2 = np.concatenate([cosT, cosT], axis=1)      # [32, T] batch-major
    r01_2 = np.concatenate([r01T, r01T], axis=1)
    r10_2 = np.concatenate([r10T, r10T], axis=1)
    ropeA_np = np.concatenate([cos2, cos2, cos2, cos2], axis=0)
    ropeB_np = np.concatenate([r01_2, r10_2, r01_2, r10_2], axis=0)
    ropeAB_np = np.ascontiguousarray(
        np.concatenate([ropeA_np[:, None, :], ropeB_np[:, None, :]],
                       axis=1).reshape(LC, 2 * T).astype(bf16))

    qw_np = np.asarray(q_norm_w, np.float32)
    kw_np = np.asarray(k_norm_w, np.float32)
    Wq_s = np.asarray(Wq, np.float32) * np.tile(qw_np, H)[None, :]
    Wk_s = np.asarray(Wk, np.float32) * np.tile(kw_np, H)[None, :]
    Wv = np.asarray(Wv, np.float32)
    Wout = np.ascontiguousarray(np.asarray(Wout, np.float32).astype(bf16))

    # cols 0:4 = q-pass selector (k rows zero), cols 4:8 = k-pass selector
    winv_np = np.zeros((128, 8), np.float32)
    winv_np[0:64, 0] = 1.0 / (qw_np * qw_np)
    winv_np[64:128, 1] = 1.0 / (qw_np * qw_np)
    winv_np[0:64, 6] = 1.0 / (kw_np * kw_np)
    winv_np[64:128, 7] = 1.0 / (kw_np * kw_np)
    winv_np = winv_np.astype(bf16)

    # rope pair-swap permutation: rot[m] = raw[sigma(m)], sigma swaps 32-row
    # halves within each 64-row head group; perm[kp, m] = 1 iff kp=sigma(m)
    perm_np = np.zeros((128, 128), np.float32)
    for m in range(128):
        g = (m // 64) * 64
        r = m - g
        sig = g + (r + 32) % 64
        perm_np[sig, m] = 1.0
    perm_np = perm_np.astype(bf16)

    in_maps = []
    for g in range(NCORES):
        sl = slice(g * LC, (g + 1) * LC)
        wqkv_np = np.ascontiguousarray(np.concatenate(
            [Wq_s[:, sl], Wk_s[:, sl], Wv[:, sl]], axis=1).astype(bf16))
        in_maps.append({
            "xT": xT_np,
            "wqkv": wqkv_np,
            "wo": Wout,
            "ropeAB": ropeAB_np,
            "winv": winv_np,
            "perm": perm_np,
        })

    res = run_bass_kernel_spmd(nc, in_maps, core_ids=list(range(NCORES)))
    LAST_RESULTS = res
    # core g, eighth j holds tokens [(8j+g)*64, (8j+g+1)*64)
    out_full = np.empty((T, D), np.float32)
    for g in range(NCORES):
        og = res.results[g]["out"]
        for j in range(NCH):
            out_full[(8 * j + g) * 64:(8 * j + g + 1) * 64] = \
                og[j * 64:(j + 1) * 64]
    return np.ascontiguousarray(
        out_full.reshape(B, S, D).transpose(1, 0, 2))


# revision 39
# speedup vs baseline: 1.1024x; 1.1024x over previous
"""Trainium2 Bass kernel for nn_Attention (S=2048, B=2, D=1024, H=16, C=64).

Tensor-parallel over heads across 8 NeuronCores (2 heads/core), fully
interleaved wavefront:
  - All static inputs are pre-cast to bf16 on the host, so every load is
    a plain (non-casting) DMA and queue assignment is free.
  - p1 (projections+norm+rope, 8 chunks of 512 tokens): Wq/Wk pre-scaled
    by the RMSNorm weights on host; sumsq of the raw projection recovered
    via matmul against 1/w^2 selector columns; Sqrt runs in two batched
    regions (2 ACT table swaps total against the softmax Exp); the
    reciprocal runs on a DRAM-reshaped [64,32] view; rstd returns as a
    stride-0 broadcast DMA and is applied as the last multiply (rope
    commutes with the per-token scale); the rope pair-swap is a PE
    permutation matmul. The elementwise chain is bf16 for the DVE 2x
    mode, split across DVE and GpSimd.
  - p2 (attention, 8 query chunks of 512): scores transposed [keys, q]
    per head with K=64 contraction on PE quadrants; one [128,1024] exp
    per key block covers both heads; attn@v accumulates in PSUM with an
    appended ones column so the softmax denominator falls out.
  - The AllToAll re-shard runs at chunk granularity (8 collectives,
    destination core = (token//64) % 8), each fired right after its
    chunk; collectives are issued from the Sync queue (a collective
    trigger blocks its issuing queue until completion, so nothing
    latency-critical may sit behind it — GpSimd keeps the bounce DMAs
    and rope arithmetic instead). p3 out-projection runs per received
    eighth, two chunks after its collective fired. p1 pieces and p3
    eighths drain from an interleave queue between p2 key blocks so the
    PE stays continuously busy (p-state ramp) while ACT paces the exps.
"""

import sys

if "/opt/trn_rl_repo" not in sys.path:
    sys.path.insert(0, "/opt/trn_rl_repo")

import numpy as np
import ml_dtypes
import concourse.bass as bass
from concourse import bacc, tile, mybir
from concourse.bass_utils import run_bass_kernel_spmd
from concourse.masks import make_identity

S, B, D, H, C = 2048, 2, 1024, 16, 64
EPS = 1e-6
NCORES = 8
T = S * B                  # 4096 tokens, batch-major: t = b*S + s
LH = H // NCORES           # 2 local heads
LC = LH * C                # 128 local head columns
TCH = 512                  # p1/p2 token chunk
NCH = T // TCH             # 8
NJT = S // 128             # 16 key blocks per batch
TOK_OUT = T // NCORES      # 512 output tokens per core

F32 = mybir.dt.float32
BF16 = mybir.dt.bfloat16
AF = mybir.ActivationFunctionType

_CACHE = {}
LAST_RESULTS = None


def _build():
    nc = bacc.Bacc("TRN2", target_bir_lowering=False, debug=False,
                   num_devices=NCORES)
    xT = nc.dram_tensor("xT", [D, T], BF16, kind="ExternalInput")
    wqkv = nc.dram_tensor("wqkv", [D, 3 * LC], BF16, kind="ExternalInput")
    wo = nc.dram_tensor("wo", [H * C, D], BF16, kind="ExternalInput")
    ropeAB = nc.dram_tensor("ropeAB", [LC, 2 * T], BF16,
                            kind="ExternalInput")
    winv = nc.dram_tensor("winv", [128, 8], BF16, kind="ExternalInput")
    perm = nc.dram_tensor("perm", [128, 128], BF16, kind="ExternalInput")
    out = nc.dram_tensor("out", [TOK_OUT, D], F32, kind="ExternalOutput")

    xT4 = xT.rearrange("(a p) t -> p a t", p=128)        # [128, 8, T]
    wqkv4 = wqkv.rearrange("(a p) c -> p a c", p=128)    # [128, 8, 3*LC]
    wo4 = wo.rearrange("(a p) n -> p a n", p=128)        # [128, 8, D]
    rope4 = ropeAB.rearrange("p (j t) -> p j t", j=2)    # [128, 2, T]

    with tile.TileContext(nc) as tc:
        with (
            tc.tile_pool(name="singles", bufs=1) as singles,
            tc.tile_pool(name="xtp", bufs=4) as xtp,
            tc.tile_pool(name="workp", bufs=2) as workp,
            tc.tile_pool(name="ps1", bufs=2, space="PSUM") as ps1p,
            tc.tile_pool(name="pss", bufs=2, space="PSUM") as pssp,
            tc.tile_pool(name="pos", bufs=2, space="PSUM") as posp,
            tc.tile_pool(name="dram", bufs=1, space="DRAM") as dram,
        ):
            # ---- constants ----
            ident = singles.tile([128, 128], BF16)
            make_identity(nc, ident)
            winv_sb = singles.tile([128, 8], BF16)
            nc.gpsimd.dma_start(out=winv_sb, in_=winv[:, :])
            permb = singles.tile([128, 128], BF16)
            nc.gpsimd.dma_start(out=permb, in_=perm[:, :])
            eps128 = singles.tile([128, 1], F32)
            nc.vector.memset(eps128, EPS)
            rdr_seed = singles.tile([NCORES, 64], BF16)
            nc.vector.memset(rdr_seed, 0.0)

            # ---- weights / rope (wo deferred: only p3 needs it) ----
            wqkv_sb = singles.tile([128, 8, 3 * LC], BF16)
            nc.gpsimd.dma_start(out=wqkv_sb, in_=wqkv4)
            rope_sb = singles.tile([128, 2, T], BF16)
            nc.gpsimd.dma_start(out=rope_sb[:, :, 0:T // 2],
                                in_=rope4[:, :, 0:T // 2])
            nc.gpsimd.dma_start(out=rope_sb[:, :, T // 2:T],
                                in_=rope4[:, :, T // 2:T])
            wo_sb = singles.tile([128, 8, D], BF16)

            # ---- persistent activations ----
            qT_sb = singles.tile([128, T], BF16)   # [2 heads x 64c, t]
            k_sb = singles.tile([128, T], BF16)
            v_sb = singles.tile([128, T // 128, LH, C + 1], BF16)
            onescol = singles.tile([128, T // 128, LH, 1], F32)
            nc.vector.memset(onescol, 1.0)
            nc.vector.tensor_copy(v_sb[:, :, :, C:C + 1], onescol)

            wbin = dram.tile([NCORES, 64], BF16, tag="wbin", name="wbin")
            wbout = dram.tile([NCORES, 64], BF16, tag="wbout", name="wbout")
            bins = [dram.tile([NCORES * 128, 64], BF16, tag=f"bin{q}",
                              name=f"bin{q}") for q in range(NCH)]
            bouts = [dram.tile([NCORES * 128, 64], BF16, tag=f"bout{q}",
                               name=f"bout{q}") for q in range(NCH)]

            xts = {}
            st = {ch: {} for ch in range(NCH)}

            def load_xt(ch):
                xt = xtp.tile([128, 8, TCH], BF16, tag="xt", name=f"xt{ch}")
                nc.gpsimd.dma_start(out=xt, in_=xT4[:, :, ch * TCH:
                                                    (ch + 1) * TCH])
                xts[ch] = xt

            # ---- p1 pieces (ACT used only for exp/sqrt) ----
            def p1_q(ch):
                if ch + 3 < NCH:
                    load_xt(ch + 3)
                psq = ps1p.tile([128, TCH], F32, tag="ps1", name=f"psq{ch}")
                for a in range(8):
                    nc.tensor.matmul(psq, wqkv_sb[:, a, 0:LC],
                                     xts[ch][:, a, :],
                                     start=(a == 0), stop=(a == 7))
                qraw = workp.tile([128, TCH], BF16, tag="qraw", bufs=6,
                                  name=f"qraw{ch}")
                nc.vector.tensor_copy(qraw, psq)
                sq2 = workp.tile([128, TCH], BF16, tag="sq2",
                                 name=f"sq2_{ch}")
                nc.vector.tensor_mul(sq2, qraw, qraw)
                st[ch]["qraw"], st[ch]["sq2"] = qraw, sq2

            def p1_k(ch):
                psk = ps1p.tile([128, TCH], F32, tag="ps1", name=f"psk{ch}")
                for a in range(8):
                    nc.tensor.matmul(psk, wqkv_sb[:, a, LC:2 * LC],
                                     xts[ch][:, a, :],
                                     start=(a == 0), stop=(a == 7))
                kraw = workp.tile([128, TCH], BF16, tag="kraw", bufs=6,
                                  name=f"kraw{ch}")
                nc.vector.tensor_copy(kraw, psk)
                sqk2 = workp.tile([128, TCH], BF16, tag="sqk2",
                                  name=f"sqk2_{ch}")
                nc.vector.tensor_mul(sqk2, kraw, kraw)
                st[ch]["kraw"], st[ch]["sqk2"] = kraw, sqk2

            def p1_v(ch):
                t0 = ch * TCH
                psv = ps1p.tile([128, TCH], F32, tag="ps1", name=f"psv{ch}")
                for a in range(8):
                    nc.tensor.matmul(psv, wqkv_sb[:, a, 2 * LC:3 * LC],
                                     xts[ch][:, a, :],
                                     start=(a == 0), stop=(a == 7))
                vt = workp.tile([128, TCH], BF16, tag="vt", name=f"vt{ch}")
                nc.vector.tensor_copy(vt, psv)
                ptv = ps1p.tile([128, 4, 128], BF16, tag="ps1",
                                name=f"ptv{ch}")
                for s5 in range(TCH // 128):
                    nc.tensor.transpose(ptv[:, s5, :],
                                        vt[:, s5 * 128:(s5 + 1) * 128],
                                        ident)
                blk0 = t0 // 128
                nc.vector.tensor_copy(
                    v_sb[:, blk0:blk0 + 4, :, 0:C],
                    ptv.rearrange("p f (l c) -> p f l c", l=LH))

            def p1_ms(ch):
                # sumsq rows 0:2 = q heads, 2:4 = k heads (one bank)
                ms = ps1p.tile([4, TCH], F32, tag="ps1", name=f"ms{ch}")
                nc.tensor.matmul(ms, winv_sb[:, 0:4], st[ch]["sq2"],
                                 start=True, stop=False)
                nc.tensor.matmul(ms, winv_sb[:, 4:8], st[ch]["sqk2"],
                                 start=False, stop=True)
                msb = workp.tile([4, TCH], F32, tag="msb", bufs=6,
                                 name=f"msb{ch}")
                nc.scalar.copy(msb, ms)
                st[ch]["msb"] = msb

            def p1_sqrt(ch):
                std = workp.tile([4, TCH], F32, tag="std", bufs=6,
                                 name=f"std{ch}")
                nc.scalar.activation(std, st[ch]["msb"], AF.Sqrt,
                                     bias=eps128[0:4, :], scale=1.0 / C)
                st[ch]["std"] = std

            def p1_fin(ch):
                t0 = ch * TCH
                rdr = dram.tile([4, TCH], F32, tag="rdr", bufs=2,
                                name=f"rdr{ch}")
                nc.sync.dma_start(out=rdr, in_=st[ch]["std"])
                rsh = workp.tile([64, 4 * TCH // 64], F32, tag="rsh",
                                 name=f"rsh{ch}")
                nc.sync.dma_start(
                    out=rsh,
                    in_=bass.AP(tensor=rdr.tensor, offset=rdr.offset,
                                ap=[[4 * TCH // 64, 64], [1, 4 * TCH // 64]]))
                rshr = workp.tile([64, 4 * TCH // 64], F32, tag="rshr",
                                  name=f"rshr{ch}")
                nc.vector.reciprocal(rshr, rsh)
                rdr2 = dram.tile([4, TCH], F32, tag="rdr2", bufs=2,
                                 name=f"rdr2{ch}")
                nc.sync.dma_start(
                    out=bass.AP(tensor=rdr2.tensor, offset=rdr2.offset,
                                ap=[[4 * TCH // 64, 64], [1, 4 * TCH // 64]]),
                    in_=rshr)
                bqw = workp.tile([128, TCH], F32, tag="bqw",
                                 name=f"bqw{ch}")
                nc.sync.dma_start(
                    out=bqw,
                    in_=bass.AP(tensor=rdr2.tensor, offset=rdr2.offset,
                                ap=[[TCH, 2], [0, 64], [1, TCH]]))
                bkw = workp.tile([128, TCH], F32, tag="bkw",
                                 name=f"bkw{ch}")
                nc.sync.dma_start(
                    out=bkw,
                    in_=bass.AP(tensor=rdr2.tensor,
                                offset=rdr2.offset + 2 * TCH,
                                ap=[[TCH, 2], [0, 64], [1, TCH]]))
                # rope + deferred rstd multiply (bf16 chain)
                for which, raw, bw, dst in (
                        ("q", st[ch]["qraw"], bqw, qT_sb),
                        ("k", st[ch]["kraw"], bkw, k_sb)):
                    t1 = workp.tile([128, TCH], BF16, tag="t1",
                                    name=f"t1{which}{ch}")
                    nc.vector.tensor_mul(t1, rope_sb[:, 0, t0:t0 + TCH], raw)
                    rot = ps1p.tile([128, TCH], F32, tag="ps1",
                                    name=f"rot{which}{ch}")
                    nc.tensor.matmul(rot, permb, raw, start=True, stop=True)
                    t2 = workp.tile([128, TCH], BF16, tag="t2",
                                    name=f"t2{which}{ch}")
                    nc.vector.tensor_mul(t2, rope_sb[:, 1, t0:t0 + TCH], rot)
                    t3 = workp.tile([128, TCH], BF16, tag="t3",
                                    name=f"t3{which}{ch}")
                    nc.gpsimd.tensor_add(t3, t1, t2)
                    nc.gpsimd.tensor_mul(dst[:, t0:t0 + TCH], t3, bw)

            # =============== phase-2 =====================================
            pos_tiles = {}

            p2_exs = {}

            def p2_part(c, r0, r1, interleave, pop_every=False):
                b = c // 4
                q0 = c * TCH
                if c not in pos_tiles:
                    pos_tiles[c] = [posp.tile([C + 1, TCH], F32, tag="pos",
                                              name=f"pos{c}_{lh}")
                                    for lh in range(LH)]
                    p2_exs[c] = {}
                exs = p2_exs[c]

                def scores_exp(jt):
                    j0 = b * S + jt * 128
                    pss = pssp.tile([128, LH, TCH], F32, tag="pss",
                                    name=f"pss{c}_{jt}")
                    for lh in range(LH):
                        nc.tensor.matmul(
                            pss[:, lh, :],
                            k_sb[64 * lh:64 * lh + 64, j0:j0 + 128],
                            qT_sb[64 * lh:64 * lh + 64, q0:q0 + TCH],
                            start=True, stop=True)
                    ex = workp.tile([128, LH, TCH], BF16, tag="ex", bufs=7,
                                    name=f"ex{c}_{jt}")
                    nc.scalar.activation(ex, pss, AF.Exp, bias=0.0,
                                         scale=0.125)
                    exs[jt] = ex

                def attnv(jt):
                    j0 = b * S + jt * 128
                    for lh in range(LH):
                        nc.tensor.matmul(
                            pos_tiles[c][lh],
                            v_sb[:, j0 // 128, lh, :],
                            exs[jt][:, lh, :],
                            start=(jt == 0), stop=(jt == NJT - 1))

                for r in range(r0, r1):
                    scores_exp(2 * r)
                    scores_exp(2 * r + 1)
                    if r >= 2:
                        attnv(2 * r - 4)
                        attnv(2 * r - 3)
                    if interleave and (pop_every or r % 2 == 1):
                        interleave.pop(0)()
                if r1 < NJT // 2:
                    return
                attnv(NJT - 4)
                attnv(NJT - 3)
                attnv(NJT - 2)
                attnv(NJT - 1)
                if interleave:
                    interleave.pop(0)()
                # --- normalize + bounce ---
                den2 = workp.tile([128, 2, TCH], F32, tag="den2",
                                  name=f"den2_{c}")
                for lh in range(LH):
                    nc.vector.tensor_copy(den2[C:C + 1, lh, :],
                                          pos_tiles[c][lh][C:C + 1, :])
                ddr = dram.tile([2, TCH], F32, tag="ddr", bufs=2,
                                name=f"ddr{c}")
                nc.sync.dma_start(out=ddr, in_=den2[C:C + 1, :, :])
                dsh = workp.tile([64, 2 * TCH // 64], F32, tag="dsh",
                                 name=f"dsh{c}")
                nc.sync.dma_start(
                    out=dsh,
                    in_=bass.AP(tensor=ddr.tensor, offset=ddr.offset,
                                ap=[[2 * TCH // 64, 64], [1, 2 * TCH // 64]]))
                dshr = workp.tile([64, 2 * TCH // 64], F32, tag="dshr",
                                  name=f"dshr{c}")
                nc.vector.reciprocal(dshr, dsh)
                ddr2 = dram.tile([2, TCH], F32, tag="ddr2", bufs=2,
                                 name=f"ddr2{c}")
                nc.sync.dma_start(
                    out=bass.AP(tensor=ddr2.tensor, offset=ddr2.offset,
                                ap=[[2 * TCH // 64, 64], [1, 2 * TCH // 64]]),
                    in_=dshr)
                for lh in range(LH):
                    dnb = workp.tile([C, TCH], F32, tag="dnb",
                                     name=f"dnb{c}_{lh}")
                    nc.sync.dma_start(
                        out=dnb,
                        in_=bass.AP(tensor=ddr2.tensor,
                                    offset=ddr2.offset + lh * TCH,
                                    ap=[[0, C], [1, TCH]]))
                    attbf = workp.tile([C, TCH], BF16, tag="attbf",
                                       name=f"attbf{c}_{lh}")
                    nc.vector.tensor_mul(attbf,
                                         pos_tiles[c][lh][0:C, :], dnb)
                    # rows d*128 + 64*lh + p, 64-token slices per dest
                    nc.gpsimd.dma_start(
                        out=bass.AP(
                            tensor=bins[c].tensor,
                            offset=bins[c].offset + C * lh * 64,
                            ap=[[64, C], [128 * 64, 8], [1, 64]]),
                        in_=attbf.rearrange("p (d t) -> p d t", d=8))

            def collective(q):
                nc.gpsimd.collective_compute(
                    "AllToAll", mybir.AluOpType.bypass,
                    replica_groups=[list(range(NCORES))],
                    ins=[bins[q][:, :].opt()],
                    outs=[bouts[q][:, :].opt()])

            # =============== phase-3 (per eighth) ========================
            def p3_eighth(q):
                atta = workp.tile([128, 8, 64], BF16, tag="atta",
                                  name=f"atta{q}")
                nc.sync.dma_start(
                    out=atta,
                    in_=bouts[q].rearrange("(g p) t -> p g t", p=128))
                for nh in range(2):
                    po3 = ps1p.tile([64, 512], F32, tag="ps1",
                                    name=f"po3_{q}_{nh}")
                    for a in range(8):
                        nc.tensor.matmul(
                            po3, atta[:, a, :],
                            wo_sb[:, a, nh * 512:(nh + 1) * 512],
                            start=(a == 0), stop=(a == 7))
                    outsb = workp.tile([64, 512], F32, tag="outsb",
                                       name=f"outsb{q}_{nh}")
                    nc.vector.tensor_copy(outsb, po3)
                    nc.sync.dma_start(
                        out=out[q * 64:(q + 1) * 64,
                                nh * 512:(nh + 1) * 512],
                        in_=outsb)

            # =============== schedule ====================================
            load_xt(0)
            load_xt(1)
            load_xt(2)

            for pair in (0, 2):
                for ch in (pair, pair + 1):
                    p1_q(ch)
                    p1_k(ch)
                    p1_v(ch)
                    p1_ms(ch)
                p1_sqrt(pair)
                p1_sqrt(pair + 1)
                p1_fin(pair)
                p1_fin(pair + 1)
            # warm up the collective path: the first AllToAll pays ~35us of
            # one-time setup; burn it on a tiny dummy while p2 ramps up
            nc.sync.dma_start(out=wbin, in_=rdr_seed)
            nc.gpsimd.collective_compute(
                "AllToAll", mybir.AluOpType.bypass,
                replica_groups=[list(range(NCORES))],
                ins=[wbin[:, :].opt()], outs=[wbout[:, :].opt()])
            nc.gpsimd.dma_start(out=wo_sb, in_=wo4)  # deferred load

            ilq = []
            for pair in (4, 6):
                for ch in (pair, pair + 1):
                    ilq.append(lambda ch=ch: p1_q(ch))
                    ilq.append(lambda ch=ch: p1_k(ch))
                    ilq.append(lambda ch=ch: p1_v(ch))
                    ilq.append(lambda ch=ch: p1_ms(ch))

                def sqrt_pair(pair=pair):
                    p1_sqrt(pair)
                    p1_sqrt(pair + 1)
                ilq.append(sqrt_pair)
                ilq.append(lambda ch=pair: p1_fin(ch))
                ilq.append(lambda ch=pair + 1: p1_fin(ch))

            for c in range(NCH):
                p2_part(c, 0, NJT // 2, ilq, pop_every=(c < 4))
                collective(c)
                if c >= 2:
                    ilq.append(lambda c=c: p3_eighth(c - 2))
            while ilq:
                ilq.pop(0)()
            p3_eighth(NCH - 2)
            p3_eighth(NCH - 1)

    nc.compile()
    return nc


def kernel(x, rope_emb, Wq, Wk, Wv, q_norm_w, k_norm_w, Wout):
    global LAST_RESULTS
    if "nc" not in _CACHE:
        _CACHE["nc"] = _build()
    nc = _CACHE["nc"]
    bf16 = ml_dtypes.bfloat16

    # batch-major tokens: t = b*S + s
    x2 = np.ascontiguousarray(
        np.transpose(np.asarray(x, np.float32), (1, 0, 2)).reshape(T, D))
    xT_np = np.ascontiguousarray(x2.T.astype(bf16))

    re = np.asarray(rope_emb, np.float32)
    cosT = np.ascontiguousarray(re[:, :, 0, 0].T)    # [32, S]
    r01T = np.ascontiguousarray(re[:, :, 0, 1].T)
    r10T = np.ascontiguousarray(re[:, :, 1, 0].T)
    cos2 = np.concatenate([cosT, cosT], axis=1)      # [32, T] batch-major
    r01_2 = np.concatenate([r01T, r01T], axis=1)
    r10_2 = np.concatenate([r10T, r10T], axis=1)
    ropeA_np = np.concatenate([cos2, cos2, cos2, cos2], axis=0)
    ropeB_np = np.concatenate([r01_2, r10_2, r01_2, r10_2], axis=0)
    ropeAB_np = np.ascontiguousarray(
        np.concatenate([ropeA_np[:, None, :], ropeB_np[:, None, :]],
                       axis=1).reshape(LC, 2 * T).astype(bf16))

    qw_np = np.asarray(q_norm_w, np.float32)
    kw_np = np.asarray(k_norm_w, np.float32)
    Wq_s = np.asarray(Wq, np.float32) * np.tile(qw_np, H)[None, :]
    Wk_s = np.asarray(Wk, np.float32) * np.tile(kw_np, H)[None, :]
    Wv = np.asarray(Wv, np.float32)
    Wout = np.ascontiguousarray(np.asarray(Wout, np.float32).astype(bf16))

    # cols 0:4 = q-pass selector (k rows zero), cols 4:8 = k-pass selector
    winv_np = np.zeros((128, 8), np.float32)
    winv_np[0:64, 0] = 1.0 / (qw_np * qw_np)
    winv_np[64:128, 1] = 1.0 / (qw_np * qw_np)
    winv_np[0:64, 6] = 1.0 / (kw_np * kw_np)
    winv_np[64:128, 7] = 1.0 / (kw_np * kw_np)
    winv_np = winv_np.astype(bf16)

    # rope pair-swap permutation: rot[m] = raw[sigma(m)], sigma swaps 32-row
    # halves within each 64-row head group; perm[kp, m] = 1 iff kp=sigma(m)
    perm_np = np.zeros((128, 128), np.float32)
    for m in range(128):
        g = (m // 64) * 64
        r = m - g
        sig = g + (r + 32) % 64
        perm_np[sig, m] = 1.0
    perm_np = perm_np.astype(bf16)

    in_maps = []
    for g in range(NCORES):
        sl = slice(g * LC, (g + 1) * LC)
        wqkv_np = np.ascontiguousarray(np.concatenate(
            [Wq_s[:, sl], Wk_s[:, sl], Wv[:, sl]], axis=1).astype(bf16))
        in_maps.append({
            "xT": xT_np,
            "wqkv": wqkv_np,
            "wo": Wout,
            "ropeAB": ropeAB_np,
            "winv": winv_np,
            "perm": perm_np,
        })

    res = run_bass_kernel_spmd(nc, in_maps, core_ids=list(range(NCORES)))
    LAST_RESULTS = res
    # core g, eighth j holds tokens [(8j+g)*64, (8j+g+1)*64)
    out_full = np.empty((T, D), np.float32)
    for g in range(NCORES):
        og = res.results[g]["out"]
        for j in range(NCH):
            out_full[(8 * j + g) * 64:(8 * j + g + 1) * 64] = \
                og[j * 64:(j + 1) * 64]
    return np.ascontiguousarray(
        out_full.reshape(B, S, D).transpose(1, 0, 2))
